# revision 3
# baseline (speedup 1.0000x reference)
"""AGNNConv distributed Trainium2 kernel (8 NeuronCores).

Strategy:
  - Destination nodes are range-partitioned across the 8 cores (12500 each),
    so segment-softmax and aggregation are fully core-local (no collectives).
  - Per core, edges are bucketed by (src-table-chunk q, dst-tile t) where a
    dst-tile is 128 consecutive local destination nodes.  Buckets are padded
    to multiples of 128 "edge slots"; the slot->bucket structure is shared
    across cores (max bucket size over cores) so one SPMD graph serves all.
  - Per-edge source rows are fetched with gpsimd.dma_gather (int16 indices,
    4 table chunks of 25088 rows each).
  - Per-edge destination rows are produced on-chip: a one-hot matrix M
    (edge-slot x dst-slot, built with a tensor_scalar is_equal against an
    iota row) is transposed on TensorE and used as lhsT to select rows of a
    locally prenormalized dst tile.  The same M is the lhsT of the
    segment-sum matmul that accumulates [weighted-feature | exp] columns
    into PSUM per dst-tile.
  - Softmax uses no max-subtraction: cos in [-1,1] and beta*cos/TEMP is
    bounded, and softmax is shift-invariant, so exp(e)/sum(exp(e)) equals
    the reference exactly.
"""

import sys
import os
import numpy as np

for _p in ('/opt/trn_rl_repo',):
    if _p not in sys.path and os.path.isdir(_p):
        sys.path.insert(0, _p)

from concourse import bass, bacc, mybir
import concourse.tile as tile
from concourse.bass_utils import run_bass_kernel_spmd
from concourse.masks import make_identity

P = 128
EPS = 1e-12
TEMP = 1.0

last_exec_ns = None


def _wrap16(arr, reps=8):
    # dma_gather index layout: element i at [i % 16, i // 16], replicated to
    # all 8 groups of 16 partitions.
    w = arr.reshape(-1, 16).T
    return np.ascontiguousarray(np.tile(w, (reps, 1)))


def _host_structure(src, dst, n_nodes, n_cores, nloc, tiles, chunk, nchunks):
    """Bucket edges per core by (q, t); build shared structure + per-core
    padded index/slot streams."""
    src = np.asarray(src, dtype=np.int64)
    dst = np.asarray(dst, dtype=np.int64)
    core = dst // nloc
    nbuckets = nchunks * tiles

    per_core = []
    counts = np.zeros((n_cores, nbuckets), dtype=np.int64)
    for c in range(n_cores):
        sel = core == c
        s_c = src[sel]
        d_c = dst[sel] - c * nloc
        t_c = d_c >> 7
        slot_c = d_c & 127
        q_c = s_c // chunk
        key = q_c * tiles + t_c
        order = np.argsort(key, kind='stable')
        s_c, slot_c, key = s_c[order], slot_c[order], key[order]
        counts[c] = np.bincount(key, minlength=nbuckets)
        per_core.append((s_c, slot_c, key))

    # shared structure: per-bucket chunk count = ceil(max over cores / 128)
    bucket_chunks = (counts.max(axis=0) + P - 1) // P  # [nbuckets]
    bucket_slots = bucket_chunks * P
    bucket_off = np.zeros(nbuckets + 1, dtype=np.int64)
    np.cumsum(bucket_slots, out=bucket_off[1:])
    s_total = int(bucket_off[-1])

    gidx_streams = []
    slot_streams = []
    for c in range(n_cores):
        s_c, slot_c, key = per_core[c]
        gidx = np.zeros(s_total, dtype=np.int16)
        slots = np.full(s_total, 255.0, dtype=np.float32)
        # position of edge i (sorted by key): bucket_off[key] + rank in bucket
        cum = np.cumsum(np.bincount(key, minlength=nbuckets))
        start_in_sorted = np.concatenate([[0], cum[:-1]])
        rank = np.arange(len(key)) - start_in_sorted[key]
        pos = bucket_off[key] + rank
        q_c = s_c // chunk
        gidx[pos] = (s_c - q_c * chunk).astype(np.int16)
        slots[pos] = slot_c.astype(np.float32)
        gidx_streams.append(_wrap16(gidx))
        slot_streams.append(np.ascontiguousarray(slots.reshape(-1, P).T))

    return bucket_chunks, bucket_off, s_total, gidx_streams, slot_streams


def _build_graph(cfg, bucket_chunks, bucket_off, s_total):
    n_pad = cfg['n_pad']
    d = cfg['d']
    tiles = cfg['tiles']
    chunk = cfg['chunk']
    nchunks = cfg['nchunks']
    nloc_pad = tiles * P
    SUB = 8          # chunks per DVE sub-block
    GBLK = 16        # max chunks per dma_gather block

    f32 = mybir.dt.float32
    bf16 = mybir.dt.bfloat16
    nc = bacc.Bacc("TRN2", target_bir_lowering=False, debug=False, num_devices=8)

    feat_ext = nc.declare_dram_parameter("feat", [n_pad, d], f32, isOutput=False)
    locfeat_ext = nc.declare_dram_parameter("locfeat", [nloc_pad, d], f32, isOutput=False)
    beta_ext = nc.declare_dram_parameter("beta128", [P, 1], f32, isOutput=False)
    iota_ext = nc.declare_dram_parameter("iota128", [P, P], bf16, isOutput=False)
    gidx_ext = nc.declare_dram_parameter("gidx", [P, s_total // 16], mybir.dt.int16, isOutput=False)
    slot_ext = nc.declare_dram_parameter("slotw", [P, s_total // P], f32, isOutput=False)
    out_ext = nc.declare_dram_parameter("out", [nloc_pad, d], f32, isOutput=True)

    eq = mybir.AluOpType.is_equal
    mul = mybir.AluOpType.mult
    add = mybir.AluOpType.add
    AF = mybir.ActivationFunctionType
    AX = mybir.AxisListType

    with tile.TileContext(nc) as tc:
        with (
            tc.tile_pool(name="const", bufs=1) as cpool,
            tc.tile_pool(name="tsc", bufs=1) as tscpool,
            tc.tile_pool(name="acc", bufs=1) as accpool,
            tc.tile_pool(name="tbuild", bufs=3) as tbpool,
            tc.tile_pool(name="small", bufs=4) as smpool,
            tc.tile_pool(name="gath", bufs=3) as gpool,
            tc.tile_pool(name="mpool", bufs=2) as mpool,
            tc.tile_pool(name="mts", bufs=4) as mtspool,
            tc.tile_pool(name="prod", bufs=2) as prodpool,
            tc.tile_pool(name="xw", bufs=2) as xwpool,
            tc.tile_pool(name="idx", bufs=3) as idxpool,
            tc.tile_pool(name="psA", bufs=2, space="PSUM") as psA,      # M^T
            tc.tile_pool(name="psB", bufs=2, space="PSUM") as psB,      # D_edge
            tc.tile_pool(name="psC", bufs=3, space="PSUM") as psC,      # acc
        ):
            iota_t = cpool.tile([P, P], bf16)
            nc.sync.dma_start(out=iota_t[:], in_=iota_ext[:])
            beta_t = cpool.tile([P, 1], f32)
            nc.sync.dma_start(out=beta_t[:], in_=beta_ext[:])
            ident = cpool.tile([P, P], bf16)
            make_identity(nc, ident[:])

            tsc = tscpool.tile([P, tiles, d], bf16)      # prenormalized dst rows
            accum = accpool.tile([P, tiles, d + 1], f32)
            nc.vector.memset(accum[:], 0.0)

            # ---- phase 1: build T_scaled = feat_loc / max(||feat_loc||, EPS)
            for t in range(tiles):
                traw = tbpool.tile([P, d], f32, tag="traw")
                nc.sync.dma_start(out=traw[:], in_=locfeat_ext[t * P:(t + 1) * P, :])
                sq = tbpool.tile([P, d], f32, tag="sq")
                ld2 = smpool.tile([P, 1], f32, tag="ld2")
                nc.scalar.activation(sq[:], traw[:], AF.Square, accum_out=ld2[:])
                ld = smpool.tile([P, 1], f32, tag="ld")
                nc.scalar.activation(ld[:], ld2[:], AF.Sqrt)
                ldc = smpool.tile([P, 1], f32, tag="ldc")
                nc.vector.tensor_scalar_max(out=ldc[:], in0=ld[:], scalar1=EPS)
                ild = smpool.tile([P, 1], f32, tag="ild")
                nc.vector.reciprocal(ild[:], ldc[:])
                nc.vector.tensor_scalar_mul(out=tsc[:, t, :], in0=traw[:], scalar1=ild[:])

            # ---- phase 2: edge stream
            # chunk -> bucket map from shared structure
            nbuckets = nchunks * tiles
            for q in range(nchunks):
                q_first_bucket = q * tiles
                q_start = int(bucket_off[q_first_bucket])
                q_end = int(bucket_off[(q + 1) * tiles])
                q_nch = (q_end - q_start) // P
                # chunk-global -> (bucket, first?, last?) within this q
                chunk_bucket = []
                for t in range(tiles):
                    b = q_first_bucket + t
                    for j in range(int(bucket_chunks[b])):
                        chunk_bucket.append((b, j == 0, j == int(bucket_chunks[b]) - 1))
                assert len(chunk_bucket) == q_nch

                acc_ps = None
                for blk0 in range(0, q_nch, GBLK):
                    nch = min(GBLK, q_nch - blk0)
                    base_slot = q_start + blk0 * P

                    idx_t = idxpool.tile([P, GBLK * 8], mybir.dt.int16, tag="idx")
                    nc.sync.dma_start(
                        out=idx_t[:, :nch * 8],
                        in_=gidx_ext[:, base_slot // 16:(base_slot + nch * P) // 16])
                    slot_t = idxpool.tile([P, GBLK], f32, tag="slot")
                    nc.sync.dma_start(
                        out=slot_t[:, :nch],
                        in_=slot_ext[:, base_slot // P:base_slot // P + nch])

                    g = gpool.tile([P, GBLK, d], f32, tag="g")
                    nc.gpsimd.dma_gather(
                        out_ap=g[:, :nch, :],
                        in_ap=feat_ext[q * chunk:(q + 1) * chunk, :],
                        idxs_ap=idx_t[:, :nch * 8],
                        num_idxs=nch * P,
                        num_idxs_reg=nch * P,
                        elem_size=d,
                        single_packet=False,
                    )

                    m_t = mpool.tile([P, GBLK, P], bf16, tag="m")
                    for j in range(nch):
                        nc.vector.tensor_scalar(
                            out=m_t[:, j, :], in0=iota_t[:],
                            scalar1=slot_t[:, j:j + 1], scalar2=None, op0=eq)

                    # sub-blocks of SUB chunks
                    for sb0 in range(0, nch, SUB):
                        nsb = min(SUB, nch - sb0)
                        dps = psB.tile([P, SUB, d], f32, tag="dps")
                        for j in range(nsb):
                            cgl = blk0 + sb0 + j
                            b, _, _ = chunk_bucket[cgl]
                            t = b - q_first_bucket
                            mtp = psA.tile([P, P], bf16, tag="mtp")
                            nc.tensor.transpose(mtp[:], m_t[:, sb0 + j, :], ident[:])
                            mts = mtspool.tile([P, P], bf16, tag="mts")
                            nc.scalar.activation(mts[:], mtp[:], AF.Copy)
                            nc.tensor.matmul(
                                dps[:, j, :], lhsT=mts[:], rhs=tsc[:, t, :],
                                start=True, stop=True)

                        S_sub = g[:, sb0:sb0 + nsb, :]
                        ssq = prodpool.tile([P, SUB, d], f32, tag="ssq")
                        nc.vector.tensor_tensor(out=ssq[:, :nsb, :], in0=S_sub, in1=S_sub, op=mul)
                        ls2 = smpool.tile([P, SUB], f32, tag="ls2")
                        nc.vector.tensor_reduce(
                            out=ls2[:, :nsb], in_=ssq[:, :nsb, :], axis=AX.X, op=add)
                        sdp = prodpool.tile([P, SUB, d], f32, tag="sdp")
                        nc.vector.tensor_tensor(
                            out=sdp[:, :nsb, :], in0=S_sub, in1=dps[:, :nsb, :], op=mul)
                        cosn = smpool.tile([P, SUB], f32, tag="cosn")
                        nc.vector.tensor_reduce(
                            out=cosn[:, :nsb], in_=sdp[:, :nsb, :], axis=AX.X, op=add)
                        ls = smpool.tile([P, SUB], f32, tag="ls")
                        nc.scalar.activation(ls[:, :nsb], ls2[:, :nsb], AF.Sqrt)
                        lsc = smpool.tile([P, SUB], f32, tag="lsc")
                        nc.vector.tensor_scalar_max(out=lsc[:, :nsb], in0=ls[:, :nsb], scalar1=EPS)
                        ils = smpool.tile([P, SUB], f32, tag="ils")
                        nc.vector.reciprocal(ils[:, :nsb], lsc[:, :nsb])
                        lg = smpool.tile([P, SUB], f32, tag="lg")
                        nc.vector.tensor_tensor(
                            out=lg[:, :nsb], in0=cosn[:, :nsb], in1=ils[:, :nsb], op=mul)
                        pt = smpool.tile([P, SUB], f32, tag="pt")
                        nc.scalar.activation(
                            pt[:, :nsb], lg[:, :nsb], AF.Exp,
                            scale=beta_t[:, 0:1])
                        xw = xwpool.tile([P, SUB, d + 1], bf16, tag="xw")
                        nc.vector.tensor_tensor(
                            out=xw[:, :nsb, 0:d], in0=S_sub,
                            in1=pt[:, :nsb, None].to_broadcast([P, nsb, d]), op=mul)
                        nc.vector.tensor_copy(out=xw[:, :nsb, d], in_=pt[:, :nsb])

                        # scatter matmuls
                        for j in range(nsb):
                            cgl = blk0 + sb0 + j
                            b, first, last = chunk_bucket[cgl]
                            t = b - q_first_bucket
                            if first:
                                acc_ps = psC.tile([P, d + 1], f32, tag="accps")
                            nc.tensor.matmul(
                                acc_ps[:], lhsT=m_t[:, sb0 + j, :],
                                rhs=xw[:, j, :], start=first, stop=last)
                            if last:
                                nc.vector.tensor_tensor(
                                    out=accum[:, t, :], in0=accum[:, t, :],
                                    in1=acc_ps[:], op=add)

            # ---- phase 3: normalize + writeback
            for t in range(tiles):
                dmx = smpool.tile([P, 1], f32, tag="dmx")
                nc.vector.tensor_scalar_max(
                    out=dmx[:], in0=accum[:, t, d:d + 1], scalar1=EPS)
                r = smpool.tile([P, 1], f32, tag="r")
                nc.vector.reciprocal(r[:], dmx[:])
                ostg = tbpool.tile([P, d], f32, tag="ostg")
                nc.vector.tensor_scalar_mul(out=ostg[:], in0=accum[:, t, 0:d], scalar1=r[:])
                nc.sync.dma_start(out=out_ext[t * P:(t + 1) * P, :], in_=ostg[:])

    nc.compile()
    return nc


def _run(feat, beta, src, dst, cfg):
    global last_exec_ns
    n = cfg['n']
    n_pad = cfg['n_pad']
    d = cfg['d']
    n_cores = cfg['n_cores']
    nloc = cfg['nloc']
    tiles = cfg['tiles']
    chunk = cfg['chunk']
    nchunks = cfg['nchunks']
    nloc_pad = tiles * P

    feat = np.ascontiguousarray(np.asarray(feat, dtype=np.float32))
    beta = np.asarray(beta, dtype=np.float32)

    bucket_chunks, bucket_off, s_total, gidx_streams, slot_streams = _host_structure(
        src, dst, n, n_cores, nloc, tiles, chunk, nchunks)

    nc = _build_graph(cfg, bucket_chunks, bucket_off, s_total)

    feat_pad = np.zeros((n_pad, d), dtype=np.float32)
    feat_pad[:n] = feat
    beta128 = np.full((P, 1), beta.reshape(-1)[0], dtype=np.float32)
    import ml_dtypes
    iota128 = np.broadcast_to(np.arange(P).astype(ml_dtypes.bfloat16), (P, P)).copy()

    in_maps = []
    for c in range(n_cores):
        lo = c * nloc
        locfeat = feat_pad[lo:lo + nloc_pad]
        if locfeat.shape[0] < nloc_pad:
            locfeat = np.vstack([locfeat, np.zeros((nloc_pad - locfeat.shape[0], d), np.float32)])
        in_maps.append({
            "feat": feat_pad,
            "locfeat": np.ascontiguousarray(locfeat),
            "beta128": beta128,
            "iota128": iota128,
            "gidx": gidx_streams[c],
            "slotw": slot_streams[c],
        })

    res = run_bass_kernel_spmd(nc, in_maps, core_ids=list(range(n_cores)),
                               trace=cfg.get('trace', False))
    last_exec_ns = res.exec_time_ns

    out = np.empty((n, d), dtype=np.float32)
    for c in range(n_cores):
        lo = c * nloc
        hi = min(lo + nloc, n)
        out[lo:hi] = res.results[c]["out"][:hi - lo]
    return out


FULL_CFG = dict(n=100000, n_pad=100352, d=64, n_cores=8, nloc=12500,
                tiles=98, chunk=25088, nchunks=4)


def kernel(feat, beta, src, dst):
    return _run(feat, beta, src, dst, dict(FULL_CFG))


# revision 8
# speedup vs baseline: 1.1219x; 1.1219x over previous
"""AGNNConv distributed Trainium2 kernel (8 NeuronCores).

Strategy:
  - Destination nodes are range-partitioned across the 8 cores (12500 each),
    so segment-softmax and aggregation are fully core-local (no collectives).
  - Per core, edges are bucketed by (src-table-chunk q, dst-tile t) where a
    dst-tile is 128 consecutive local destination nodes.  Buckets are padded
    to multiples of 128 "edge slots"; the slot->bucket structure is shared
    across cores (max bucket size over cores) so one SPMD graph serves all.
  - Per-edge source rows are fetched with gpsimd.dma_gather (int16 indices,
    4 table chunks of 25088 rows each).
  - Per-edge destination rows are produced on-chip: a one-hot matrix M
    (edge-slot x dst-slot, built with a tensor_scalar is_equal against an
    iota row) is transposed on TensorE and used as lhsT to select rows of a
    locally prenormalized dst tile.  The same M is the lhsT of the
    segment-sum matmul that accumulates [weighted-feature | exp] columns
    into PSUM per dst-tile.
  - Softmax uses no max-subtraction: cos in [-1,1] and beta*cos/TEMP is
    bounded, and softmax is shift-invariant, so exp(e)/sum(exp(e)) equals
    the reference exactly.
"""

import sys
import os
import numpy as np

for _p in ('/opt/trn_rl_repo',):
    if _p not in sys.path and os.path.isdir(_p):
        sys.path.insert(0, _p)

from concourse import bass, bacc, mybir
import concourse.tile as tile
from concourse.bass_utils import run_bass_kernel_spmd
from concourse.masks import make_identity

P = 128
EPS = 1e-12
TEMP = 1.0

last_exec_ns = None


def _wrap16(arr, reps=8):
    # dma_gather index layout: element i at [i % 16, i // 16], replicated to
    # all 8 groups of 16 partitions.
    w = arr.reshape(-1, 16).T
    return np.ascontiguousarray(np.tile(w, (reps, 1)))


def _host_structure(src, dst, n_nodes, n_cores, nloc, tiles, chunk, nchunks):
    """Bucket edges per core by (q, t); build shared structure + per-core
    padded index/slot streams."""
    src = np.asarray(src, dtype=np.int64)
    dst = np.asarray(dst, dtype=np.int64)
    core = dst // nloc
    nbuckets = nchunks * tiles

    per_core = []
    counts = np.zeros((n_cores, nbuckets), dtype=np.int64)
    for c in range(n_cores):
        sel = core == c
        s_c = src[sel]
        d_c = dst[sel] - c * nloc
        t_c = d_c >> 7
        slot_c = d_c & 127
        q_c = s_c // chunk
        key = q_c * tiles + t_c
        order = np.argsort(key, kind='stable')
        s_c, slot_c, key = s_c[order], slot_c[order], key[order]
        counts[c] = np.bincount(key, minlength=nbuckets)
        per_core.append((s_c, slot_c, key))

    # shared structure: per-bucket chunk count = ceil(max over cores / 128)
    bucket_chunks = (counts.max(axis=0) + P - 1) // P  # [nbuckets]
    bucket_slots = bucket_chunks * P
    bucket_off = np.zeros(nbuckets + 1, dtype=np.int64)
    np.cumsum(bucket_slots, out=bucket_off[1:])
    s_total = int(bucket_off[-1])

    gidx_streams = []
    slot_streams = []
    for c in range(n_cores):
        s_c, slot_c, key = per_core[c]
        gidx = np.zeros(s_total, dtype=np.int16)
        slots = np.full(s_total, 255.0, dtype=np.float32)
        # position of edge i (sorted by key): bucket_off[key] + rank in bucket
        cum = np.cumsum(np.bincount(key, minlength=nbuckets))
        start_in_sorted = np.concatenate([[0], cum[:-1]])
        rank = np.arange(len(key)) - start_in_sorted[key]
        pos = bucket_off[key] + rank
        q_c = s_c // chunk
        gidx[pos] = (s_c - q_c * chunk).astype(np.int16)
        slots[pos] = slot_c.astype(np.float32)
        gidx_streams.append(_wrap16(gidx))
        import ml_dtypes
        slot_streams.append(np.ascontiguousarray(slots.reshape(-1, P).T.astype(ml_dtypes.bfloat16)))

    return bucket_chunks, bucket_off, s_total, gidx_streams, slot_streams


def _build_graph(cfg, bucket_chunks, bucket_off, s_total):
    n_pad = cfg['n_pad']
    d = cfg['d']
    tiles = cfg['tiles']
    chunk = cfg['chunk']
    nchunks = cfg['nchunks']
    nloc_pad = tiles * P
    SUB = 16         # chunks per DVE sub-block
    GBLK = 16        # max chunks per dma_gather block

    f32 = mybir.dt.float32
    bf16 = mybir.dt.bfloat16
    nc = bacc.Bacc("TRN2", target_bir_lowering=False, debug=False, num_devices=8)

    feat_ext = nc.declare_dram_parameter("feat", [n_pad, d], f32, isOutput=False)
    locfeat_ext = nc.declare_dram_parameter("locfeat", [nloc_pad, d], f32, isOutput=False)
    beta_ext = nc.declare_dram_parameter("beta128", [P, 1], f32, isOutput=False)
    iota_ext = nc.declare_dram_parameter("iota128", [P, P], bf16, isOutput=False)
    gidx_ext = nc.declare_dram_parameter("gidx", [P, s_total // 16], mybir.dt.int16, isOutput=False)
    slot_ext = nc.declare_dram_parameter("slotw", [P, s_total // P], bf16, isOutput=False)
    out_ext = nc.declare_dram_parameter("out", [nloc_pad, d], f32, isOutput=True)

    eq = mybir.AluOpType.is_equal
    mul = mybir.AluOpType.mult
    add = mybir.AluOpType.add
    AF = mybir.ActivationFunctionType
    AX = mybir.AxisListType

    with tile.TileContext(nc) as tc:
        with (
            tc.tile_pool(name="const", bufs=1) as cpool,
            tc.tile_pool(name="tsc", bufs=1) as tscpool,
            tc.tile_pool(name="acc", bufs=1) as accpool,
            tc.tile_pool(name="tbuild", bufs=3) as tbpool,
            tc.tile_pool(name="small", bufs=4) as smpool,
            tc.tile_pool(name="gath", bufs=3) as gpool,
            tc.tile_pool(name="mpool", bufs=2) as mpool,
            tc.tile_pool(name="mts", bufs=4) as mtspool,
            tc.tile_pool(name="prod", bufs=2) as prodpool,
            tc.tile_pool(name="xw", bufs=2) as xwpool,
            tc.tile_pool(name="idx", bufs=3) as idxpool,
            tc.tile_pool(name="psA", bufs=2, space="PSUM") as psA,      # M^T
            tc.tile_pool(name="psB", bufs=2, space="PSUM") as psB,      # D_edge
            tc.tile_pool(name="psC", bufs=2, space="PSUM") as psC,      # acc
        ):
            iota_t = cpool.tile([P, P], bf16)
            nc.sync.dma_start(out=iota_t[:], in_=iota_ext[:])
            beta_t = cpool.tile([P, 1], f32)
            nc.sync.dma_start(out=beta_t[:], in_=beta_ext[:])
            ident = cpool.tile([P, P], bf16)
            make_identity(nc, ident[:])

            tsc = tscpool.tile([P, tiles, d], bf16)      # prenormalized dst rows
            accum = accpool.tile([P, tiles, d + 1], f32)
            nc.vector.memset(accum[:], 0.0)

            # ---- phase 1: build T_scaled = feat_loc / max(||feat_loc||, EPS)
            for t in range(tiles):
                traw = tbpool.tile([P, d], f32, tag="traw")
                nc.sync.dma_start(out=traw[:], in_=locfeat_ext[t * P:(t + 1) * P, :])
                sq = tbpool.tile([P, d], f32, tag="sq")
                ld2 = smpool.tile([P, 1], f32, tag="ld2")
                nc.scalar.activation(sq[:], traw[:], AF.Square, accum_out=ld2[:])
                ld = smpool.tile([P, 1], f32, tag="ld")
                nc.scalar.activation(ld[:], ld2[:], AF.Sqrt)
                ldc = smpool.tile([P, 1], f32, tag="ldc")
                nc.vector.tensor_scalar_max(out=ldc[:], in0=ld[:], scalar1=EPS)
                ild = smpool.tile([P, 1], f32, tag="ild")
                nc.vector.reciprocal(ild[:], ldc[:])
                nc.vector.tensor_scalar_mul(out=tsc[:, t, :], in0=traw[:], scalar1=ild[:])

            # ---- phase 2: edge stream
            # chunk -> bucket map from shared structure
            nbuckets = nchunks * tiles
            for q in range(nchunks):
                q_first_bucket = q * tiles
                q_start = int(bucket_off[q_first_bucket])
                q_end = int(bucket_off[(q + 1) * tiles])
                q_nch = (q_end - q_start) // P
                # chunk-global -> (bucket, first?, last?) within this q
                chunk_bucket = []
                for t in range(tiles):
                    b = q_first_bucket + t
                    for j in range(int(bucket_chunks[b])):
                        chunk_bucket.append((b, j == 0, j == int(bucket_chunks[b]) - 1))
                assert len(chunk_bucket) == q_nch

                acc_ps = None
                for blk0 in range(0, q_nch, GBLK):
                    nch = min(GBLK, q_nch - blk0)
                    base_slot = q_start + blk0 * P

                    idx_t = idxpool.tile([P, GBLK * 8], mybir.dt.int16, tag="idx")
                    nc.sync.dma_start(
                        out=idx_t[:, :nch * 8],
                        in_=gidx_ext[:, base_slot // 16:(base_slot + nch * P) // 16])
                    slot_t = idxpool.tile([P, GBLK], bf16, tag="slot")
                    nc.sync.dma_start(
                        out=slot_t[:, :nch],
                        in_=slot_ext[:, base_slot // P:base_slot // P + nch])

                    g = gpool.tile([P, GBLK, d], f32, tag="g")
                    nc.gpsimd.dma_gather(
                        out_ap=g[:, :nch, :],
                        in_ap=feat_ext[q * chunk:(q + 1) * chunk, :],
                        idxs_ap=idx_t[:, :nch * 8],
                        num_idxs=nch * P,
                        num_idxs_reg=nch * P,
                        elem_size=d,
                        single_packet=False,
                    )

                    m_t = mpool.tile([P, GBLK, P], bf16, tag="m")
                    nc.vector.tensor_tensor(
                        out=m_t[:, :nch, :],
                        in0=slot_t[:, :nch, None].to_broadcast([P, nch, P]),
                        in1=iota_t[:, None, :].to_broadcast([P, nch, P]),
                        op=eq)

                    # sub-blocks of SUB chunks
                    for sb0 in range(0, nch, SUB):
                        nsb = min(SUB, nch - sb0)
                        dps = psB.tile([P, SUB, d], f32, tag="dps")
                        for g0 in range(0, nsb, 4):
                            ng = min(4, nsb - g0)
                            mtp = psA.tile([P, 4, P], bf16, tag="mtp")
                            for j in range(g0, g0 + ng):
                                nc.tensor.transpose(
                                    mtp[:, j - g0, :], m_t[:, sb0 + j, :], ident[:])
                            mts = mtspool.tile([P, 4, P], bf16, tag="mts")
                            nc.scalar.activation(
                                mts[:, :ng, :], mtp[:, :ng, :], AF.Copy)
                            for j in range(g0, g0 + ng):
                                cgl = blk0 + sb0 + j
                                b, _, _ = chunk_bucket[cgl]
                                t = b - q_first_bucket
                                nc.tensor.matmul(
                                    dps[:, j, :], lhsT=mts[:, j - g0, :],
                                    rhs=tsc[:, t, :], start=True, stop=True)

                        S_sub = g[:, sb0:sb0 + nsb, :]
                        ssq = prodpool.tile([P, SUB, d], f32, tag="ssq")
                        nc.vector.tensor_tensor(out=ssq[:, :nsb, :], in0=S_sub, in1=S_sub, op=mul)
                        ls2 = smpool.tile([P, SUB], f32, tag="ls2")
                        nc.vector.tensor_reduce(
                            out=ls2[:, :nsb], in_=ssq[:, :nsb, :], axis=AX.X, op=add)
                        sdp = prodpool.tile([P, SUB, d], f32, tag="sdp")
                        nc.vector.tensor_tensor(
                            out=sdp[:, :nsb, :], in0=S_sub, in1=dps[:, :nsb, :], op=mul)
                        cosn = smpool.tile([P, SUB], f32, tag="cosn")
                        nc.vector.tensor_reduce(
                            out=cosn[:, :nsb], in_=sdp[:, :nsb, :], axis=AX.X, op=add)
                        ls = smpool.tile([P, SUB], f32, tag="ls")
                        nc.scalar.activation(ls[:, :nsb], ls2[:, :nsb], AF.Sqrt)
                        lsc = smpool.tile([P, SUB], f32, tag="lsc")
                        nc.vector.tensor_scalar_max(out=lsc[:, :nsb], in0=ls[:, :nsb], scalar1=EPS)
                        ils = smpool.tile([P, SUB], f32, tag="ils")
                        nc.vector.reciprocal(ils[:, :nsb], lsc[:, :nsb])
                        lg = smpool.tile([P, SUB], f32, tag="lg")
                        nc.vector.tensor_tensor(
                            out=lg[:, :nsb], in0=cosn[:, :nsb], in1=ils[:, :nsb], op=mul)
                        pt = smpool.tile([P, SUB], f32, tag="pt")
                        nc.scalar.activation(
                            pt[:, :nsb], lg[:, :nsb], AF.Exp,
                            scale=beta_t[:, 0:1])
                        xw = xwpool.tile([P, SUB, d + 1], bf16, tag="xw")
                        nc.vector.tensor_tensor(
                            out=xw[:, :nsb, 0:d], in0=S_sub,
                            in1=pt[:, :nsb, None].to_broadcast([P, nsb, d]), op=mul)
                        nc.vector.tensor_copy(out=xw[:, :nsb, d], in_=pt[:, :nsb])

                        # scatter matmuls
                        for j in range(nsb):
                            cgl = blk0 + sb0 + j
                            b, first, last = chunk_bucket[cgl]
                            t = b - q_first_bucket
                            if first:
                                acc_ps = psC.tile([P, d + 1], f32, tag="accps")
                            nc.tensor.matmul(
                                acc_ps[:], lhsT=m_t[:, sb0 + j, :],
                                rhs=xw[:, j, :], start=first, stop=last)
                            if last:
                                nc.vector.tensor_tensor(
                                    out=accum[:, t, :], in0=accum[:, t, :],
                                    in1=acc_ps[:], op=add)

            # ---- phase 3: normalize + writeback
            for t in range(tiles):
                dmx = smpool.tile([P, 1], f32, tag="dmx")
                nc.vector.tensor_scalar_max(
                    out=dmx[:], in0=accum[:, t, d:d + 1], scalar1=EPS)
                r = smpool.tile([P, 1], f32, tag="r")
                nc.vector.reciprocal(r[:], dmx[:])
                ostg = tbpool.tile([P, d], f32, tag="ostg")
                nc.vector.tensor_scalar_mul(out=ostg[:], in0=accum[:, t, 0:d], scalar1=r[:])
                nc.sync.dma_start(out=out_ext[t * P:(t + 1) * P, :], in_=ostg[:])

    nc.compile()
    return nc


def _run(feat, beta, src, dst, cfg):
    global last_exec_ns
    n = cfg['n']
    n_pad = cfg['n_pad']
    d = cfg['d']
    n_cores = cfg['n_cores']
    nloc = cfg['nloc']
    tiles = cfg['tiles']
    chunk = cfg['chunk']
    nchunks = cfg['nchunks']
    nloc_pad = tiles * P

    feat = np.ascontiguousarray(np.asarray(feat, dtype=np.float32))
    beta = np.asarray(beta, dtype=np.float32)

    bucket_chunks, bucket_off, s_total, gidx_streams, slot_streams = _host_structure(
        src, dst, n, n_cores, nloc, tiles, chunk, nchunks)

    nc = _build_graph(cfg, bucket_chunks, bucket_off, s_total)

    feat_pad = np.zeros((n_pad, d), dtype=np.float32)
    feat_pad[:n] = feat
    beta128 = np.full((P, 1), beta.reshape(-1)[0], dtype=np.float32)
    import ml_dtypes
    iota128 = np.broadcast_to(np.arange(P).astype(ml_dtypes.bfloat16), (P, P)).copy()

    in_maps = []
    for c in range(n_cores):
        lo = c * nloc
        locfeat = feat_pad[lo:lo + nloc_pad]
        if locfeat.shape[0] < nloc_pad:
            locfeat = np.vstack([locfeat, np.zeros((nloc_pad - locfeat.shape[0], d), np.float32)])
        in_maps.append({
            "feat": feat_pad,
            "locfeat": np.ascontiguousarray(locfeat),
            "beta128": beta128,
            "iota128": iota128,
            "gidx": gidx_streams[c],
            "slotw": slot_streams[c],
        })

    res = run_bass_kernel_spmd(nc, in_maps, core_ids=list(range(n_cores)),
                               trace=cfg.get('trace', False))
    last_exec_ns = res.exec_time_ns

    out = np.empty((n, d), dtype=np.float32)
    for c in range(n_cores):
        lo = c * nloc
        hi = min(lo + nloc, n)
        out[lo:hi] = res.results[c]["out"][:hi - lo]
    return out


FULL_CFG = dict(n=100000, n_pad=100352, d=64, n_cores=8, nloc=12500,
                tiles=98, chunk=25088, nchunks=4)


def kernel(feat, beta, src, dst):
    return _run(feat, beta, src, dst, dict(FULL_CFG))


# revision 10
# speedup vs baseline: 1.7786x; 1.5853x over previous
"""AGNNConv distributed Trainium2 kernel (8 NeuronCores).

Strategy:
  - Destination nodes are range-partitioned across the 8 cores (12500 each),
    so segment-softmax and aggregation are fully core-local (no collectives).
  - Per core, edges are bucketed by (src-table-chunk q, dst-tile t) where a
    dst-tile is 128 consecutive local destination nodes.  Buckets are padded
    to multiples of 128 "edge slots"; the slot->bucket structure is shared
    across cores (max bucket size over cores) so one SPMD graph serves all.
  - Per-edge source rows are fetched with gpsimd.dma_gather (int16 indices,
    4 table chunks of 25088 rows each).
  - Per-edge destination rows are produced on-chip: a one-hot matrix M
    (edge-slot x dst-slot, built with a tensor_scalar is_equal against an
    iota row) is transposed on TensorE and used as lhsT to select rows of a
    locally prenormalized dst tile.  The same M is the lhsT of the
    segment-sum matmul that accumulates [weighted-feature | exp] columns
    into PSUM per dst-tile.
  - Softmax uses no max-subtraction: cos in [-1,1] and beta*cos/TEMP is
    bounded, and softmax is shift-invariant, so exp(e)/sum(exp(e)) equals
    the reference exactly.
"""

import sys
import os
import numpy as np

for _p in ('/opt/trn_rl_repo',):
    if _p not in sys.path and os.path.isdir(_p):
        sys.path.insert(0, _p)

from concourse import bass, bacc, mybir
import concourse.tile as tile
from concourse.bass_utils import run_bass_kernel_spmd
from concourse.masks import make_identity

P = 128
EPS = 1e-12
TEMP = 1.0

last_exec_ns = None


def _wrap16(arr, reps=8):
    # dma_gather index layout: element i at [i % 16, i // 16], replicated to
    # all 8 groups of 16 partitions.
    w = arr.reshape(-1, 16).T
    return np.ascontiguousarray(np.tile(w, (reps, 1)))


def _host_structure(src, dst, n_nodes, n_cores, nloc, tiles, chunk, nchunks):
    """Bucket edges per core by (q, t); build shared structure + per-core
    padded index/slot streams."""
    src = np.asarray(src, dtype=np.int64)
    dst = np.asarray(dst, dtype=np.int64)
    core = dst // nloc
    nbuckets = nchunks * tiles

    per_core = []
    counts = np.zeros((n_cores, nbuckets), dtype=np.int64)
    for c in range(n_cores):
        sel = core == c
        s_c = src[sel]
        d_c = dst[sel] - c * nloc
        t_c = d_c >> 7
        slot_c = d_c & 127
        q_c = s_c // chunk
        key = q_c * tiles + t_c
        order = np.argsort(key, kind='stable')
        s_c, slot_c, key = s_c[order], slot_c[order], key[order]
        counts[c] = np.bincount(key, minlength=nbuckets)
        per_core.append((s_c, slot_c, key))

    # shared structure: per-bucket chunk count = ceil(max over cores / 128)
    bucket_chunks = (counts.max(axis=0) + P - 1) // P  # [nbuckets]
    bucket_slots = bucket_chunks * P
    bucket_off = np.zeros(nbuckets + 1, dtype=np.int64)
    np.cumsum(bucket_slots, out=bucket_off[1:])
    s_total = int(bucket_off[-1])

    gidx_streams = []
    slot_streams = []
    for c in range(n_cores):
        s_c, slot_c, key = per_core[c]
        gidx = np.zeros(s_total, dtype=np.int16)
        slots = np.full(s_total, 255.0, dtype=np.float32)
        # position of edge i (sorted by key): bucket_off[key] + rank in bucket
        cum = np.cumsum(np.bincount(key, minlength=nbuckets))
        start_in_sorted = np.concatenate([[0], cum[:-1]])
        rank = np.arange(len(key)) - start_in_sorted[key]
        pos = bucket_off[key] + rank
        q_c = s_c // chunk
        gidx[pos] = (s_c - q_c * chunk).astype(np.int16)
        slots[pos] = slot_c.astype(np.float32)
        gidx_streams.append(_wrap16(gidx))
        import ml_dtypes
        slot_streams.append(np.ascontiguousarray(slots.reshape(-1, P).T.astype(ml_dtypes.bfloat16)))

    return bucket_chunks, bucket_off, s_total, gidx_streams, slot_streams


def _build_graph(cfg, bucket_chunks, bucket_off, s_total):
    n_pad = cfg['n_pad']
    d = cfg['d']
    tiles = cfg['tiles']
    chunk = cfg['chunk']
    nchunks = cfg['nchunks']
    nloc_pad = tiles * P
    SUB = 16         # chunks per DVE sub-block
    GBLK = 16        # max chunks per dma_gather block

    f32 = mybir.dt.float32
    bf16 = mybir.dt.bfloat16
    nc = bacc.Bacc("TRN2", target_bir_lowering=False, debug=False, num_devices=8)

    feat_ext = nc.declare_dram_parameter("feat", [n_pad, d], f32, isOutput=False)
    locfeat_ext = nc.declare_dram_parameter("locfeat", [nloc_pad, d], f32, isOutput=False)
    beta_ext = nc.declare_dram_parameter("beta128", [P, 1], f32, isOutput=False)
    iota_ext = nc.declare_dram_parameter("iota128", [P, P], bf16, isOutput=False)
    gidx_ext = nc.declare_dram_parameter("gidx", [P, s_total // 16], mybir.dt.int16, isOutput=False)
    slot_ext = nc.declare_dram_parameter("slotw", [P, s_total // P], bf16, isOutput=False)
    out_ext = nc.declare_dram_parameter("out", [nloc_pad, d], f32, isOutput=True)

    eq = mybir.AluOpType.is_equal
    mul = mybir.AluOpType.mult
    add = mybir.AluOpType.add
    AF = mybir.ActivationFunctionType
    AX = mybir.AxisListType

    with tile.TileContext(nc) as tc:
        with (
            tc.tile_pool(name="const", bufs=1) as cpool,
            tc.tile_pool(name="tsc", bufs=1) as tscpool,
            tc.tile_pool(name="acc", bufs=1) as accpool,
            tc.tile_pool(name="tbuild", bufs=3) as tbpool,
            tc.tile_pool(name="small", bufs=6) as smpool,
            tc.tile_pool(name="gath", bufs=4) as gpool,
            tc.tile_pool(name="mpool", bufs=3) as mpool,
            tc.tile_pool(name="mts", bufs=4) as mtspool,
            tc.tile_pool(name="prod", bufs=3) as prodpool,
            tc.tile_pool(name="xw", bufs=3) as xwpool,
            tc.tile_pool(name="idx", bufs=4) as idxpool,
            tc.tile_pool(name="psA", bufs=2, space="PSUM") as psA,      # M^T
            tc.tile_pool(name="psB", bufs=3, space="PSUM") as psB,      # D_edge
            tc.tile_pool(name="psC", bufs=3, space="PSUM") as psC,      # acc
        ):
            iota_t = cpool.tile([P, P], bf16)
            nc.sync.dma_start(out=iota_t[:], in_=iota_ext[:])
            beta_t = cpool.tile([P, 1], f32)
            nc.sync.dma_start(out=beta_t[:], in_=beta_ext[:])
            ident = cpool.tile([P, P], bf16)
            make_identity(nc, ident[:])

            tsc = tscpool.tile([P, tiles, d], bf16)      # prenormalized dst rows
            accum = accpool.tile([P, tiles, d + 1], f32)
            nc.vector.memset(accum[:], 0.0)

            # ---- phase 1: build T_scaled = feat_loc / max(||feat_loc||, EPS)
            for t in range(tiles):
                traw = tbpool.tile([P, d], f32, tag="traw")
                nc.sync.dma_start(out=traw[:], in_=locfeat_ext[t * P:(t + 1) * P, :])
                sq = tbpool.tile([P, d], f32, tag="sq")
                ld2 = smpool.tile([P, 1], f32, tag="ld2")
                nc.scalar.activation(sq[:], traw[:], AF.Square, accum_out=ld2[:])
                ld = smpool.tile([P, 1], f32, tag="ld")
                nc.scalar.activation(ld[:], ld2[:], AF.Sqrt)
                ldc = smpool.tile([P, 1], f32, tag="ldc")
                nc.vector.tensor_scalar_max(out=ldc[:], in0=ld[:], scalar1=EPS)
                ild = smpool.tile([P, 1], f32, tag="ild")
                nc.vector.reciprocal(ild[:], ldc[:])
                nc.vector.tensor_scalar_mul(out=tsc[:, t, :], in0=traw[:], scalar1=ild[:])

            # ---- phase 2: edge stream
            # chunk -> bucket map from shared structure
            nbuckets = nchunks * tiles
            for q in range(nchunks):
                q_first_bucket = q * tiles
                q_start = int(bucket_off[q_first_bucket])
                q_end = int(bucket_off[(q + 1) * tiles])
                q_nch = (q_end - q_start) // P
                # chunk-global -> (bucket, first?, last?) within this q
                chunk_bucket = []
                for t in range(tiles):
                    b = q_first_bucket + t
                    for j in range(int(bucket_chunks[b])):
                        chunk_bucket.append((b, j == 0, j == int(bucket_chunks[b]) - 1))
                assert len(chunk_bucket) == q_nch

                acc_ps = None
                for blk0 in range(0, q_nch, GBLK):
                    nch = min(GBLK, q_nch - blk0)
                    base_slot = q_start + blk0 * P

                    idx_t = idxpool.tile([P, GBLK * 8], mybir.dt.int16, tag="idx")
                    nc.sync.dma_start(
                        out=idx_t[:, :nch * 8],
                        in_=gidx_ext[:, base_slot // 16:(base_slot + nch * P) // 16])
                    slot_t = idxpool.tile([P, GBLK], bf16, tag="slot")
                    nc.sync.dma_start(
                        out=slot_t[:, :nch],
                        in_=slot_ext[:, base_slot // P:base_slot // P + nch])

                    g = gpool.tile([P, GBLK, d], f32, tag="g")
                    nc.gpsimd.dma_gather(
                        out_ap=g[:, :nch, :],
                        in_ap=feat_ext[q * chunk:(q + 1) * chunk, :],
                        idxs_ap=idx_t[:, :nch * 8],
                        num_idxs=nch * P,
                        num_idxs_reg=nch * P,
                        elem_size=d,
                        single_packet=False,
                    )

                    m_t = mpool.tile([P, GBLK, P], bf16, tag="m")
                    nc.vector.tensor_tensor(
                        out=m_t[:, :nch, :],
                        in0=slot_t[:, :nch, None].to_broadcast([P, nch, P]),
                        in1=iota_t[:, None, :].to_broadcast([P, nch, P]),
                        op=eq)

                    # M^T via TensorE transposes (groups of 4) + ACT evacuation,
                    # then per-edge dst rows via M @ T_scaled into half-block
                    # PSUM tiles (1 bank each).
                    HALF = 8
                    dps_halves = []
                    for g0 in range(0, nch, 4):
                        ng = min(4, nch - g0)
                        mtp = psA.tile([P, 4, P], bf16, tag="mtp")
                        for j in range(g0, g0 + ng):
                            nc.tensor.transpose(
                                mtp[:, j - g0, :], m_t[:, j, :], ident[:])
                        mts = mtspool.tile([P, 4, P], bf16, tag="mts")
                        nc.scalar.activation(
                            mts[:, :ng, :], mtp[:, :ng, :], AF.Copy)
                        if g0 % HALF == 0:
                            dps = psB.tile([P, HALF, d], f32, tag="dps")
                            dps_halves.append(dps)
                        for j in range(g0, g0 + ng):
                            cgl = blk0 + j
                            b, _, _ = chunk_bucket[cgl]
                            t = b - q_first_bucket
                            nc.tensor.matmul(
                                dps[:, j % HALF, :], lhsT=mts[:, j - g0, :],
                                rhs=tsc[:, t, :], start=True, stop=True)

                    S_blk = g[:, :nch, :]
                    ssq = prodpool.tile([P, GBLK, d], f32, tag="ssq")
                    nc.vector.tensor_tensor(out=ssq[:, :nch, :], in0=S_blk, in1=S_blk, op=mul)
                    ls2 = smpool.tile([P, GBLK], f32, tag="ls2")
                    nc.vector.tensor_reduce(
                        out=ls2[:, :nch], in_=ssq[:, :nch, :], axis=AX.X, op=add)
                    cosn = smpool.tile([P, GBLK], f32, tag="cosn")
                    for hi, dps in enumerate(dps_halves):
                        h0 = hi * HALF
                        nh = min(HALF, nch - h0)
                        sdp = prodpool.tile([P, HALF, d], f32, tag="sdp")
                        nc.vector.tensor_tensor(
                            out=sdp[:, :nh, :], in0=g[:, h0:h0 + nh, :],
                            in1=dps[:, :nh, :], op=mul)
                        nc.vector.tensor_reduce(
                            out=cosn[:, h0:h0 + nh], in_=sdp[:, :nh, :], axis=AX.X, op=add)
                    ls = smpool.tile([P, GBLK], f32, tag="ls")
                    nc.scalar.activation(ls[:, :nch], ls2[:, :nch], AF.Sqrt)
                    lsc = smpool.tile([P, GBLK], f32, tag="lsc")
                    nc.vector.tensor_scalar_max(out=lsc[:, :nch], in0=ls[:, :nch], scalar1=EPS)
                    ils = smpool.tile([P, GBLK], f32, tag="ils")
                    nc.vector.reciprocal(ils[:, :nch], lsc[:, :nch])
                    lg = smpool.tile([P, GBLK], f32, tag="lg")
                    nc.vector.tensor_tensor(
                        out=lg[:, :nch], in0=cosn[:, :nch], in1=ils[:, :nch], op=mul)
                    pt = smpool.tile([P, GBLK], f32, tag="pt")
                    nc.scalar.activation(
                        pt[:, :nch], lg[:, :nch], AF.Exp, scale=beta_t[:, 0:1])
                    xw = xwpool.tile([P, GBLK, d + 1], bf16, tag="xw")
                    nc.vector.tensor_tensor(
                        out=xw[:, :nch, 0:d], in0=S_blk,
                        in1=pt[:, :nch, None].to_broadcast([P, nch, d]), op=mul)
                    nc.vector.tensor_copy(out=xw[:, :nch, d], in_=pt[:, :nch])

                    # scatter matmuls
                    for j in range(nch):
                        cgl = blk0 + j
                        b, first, last = chunk_bucket[cgl]
                        t = b - q_first_bucket
                        if first:
                            acc_ps = psC.tile([P, d + 1], f32, tag="accps")
                        nc.tensor.matmul(
                            acc_ps[:], lhsT=m_t[:, j, :],
                            rhs=xw[:, j, :], start=first, stop=last)
                        if last:
                            nc.vector.tensor_tensor(
                                out=accum[:, t, :], in0=accum[:, t, :],
                                in1=acc_ps[:], op=add)

            # ---- phase 3: normalize + writeback
            for t in range(tiles):
                dmx = smpool.tile([P, 1], f32, tag="dmx")
                nc.vector.tensor_scalar_max(
                    out=dmx[:], in0=accum[:, t, d:d + 1], scalar1=EPS)
                r = smpool.tile([P, 1], f32, tag="r")
                nc.vector.reciprocal(r[:], dmx[:])
                ostg = tbpool.tile([P, d], f32, tag="ostg")
                nc.vector.tensor_scalar_mul(out=ostg[:], in0=accum[:, t, 0:d], scalar1=r[:])
                nc.sync.dma_start(out=out_ext[t * P:(t + 1) * P, :], in_=ostg[:])

    nc.compile()
    return nc


def _run(feat, beta, src, dst, cfg):
    global last_exec_ns
    n = cfg['n']
    n_pad = cfg['n_pad']
    d = cfg['d']
    n_cores = cfg['n_cores']
    nloc = cfg['nloc']
    tiles = cfg['tiles']
    chunk = cfg['chunk']
    nchunks = cfg['nchunks']
    nloc_pad = tiles * P

    feat = np.ascontiguousarray(np.asarray(feat, dtype=np.float32))
    beta = np.asarray(beta, dtype=np.float32)

    bucket_chunks, bucket_off, s_total, gidx_streams, slot_streams = _host_structure(
        src, dst, n, n_cores, nloc, tiles, chunk, nchunks)

    nc = _build_graph(cfg, bucket_chunks, bucket_off, s_total)

    feat_pad = np.zeros((n_pad, d), dtype=np.float32)
    feat_pad[:n] = feat
    beta128 = np.full((P, 1), beta.reshape(-1)[0], dtype=np.float32)
    import ml_dtypes
    iota128 = np.broadcast_to(np.arange(P).astype(ml_dtypes.bfloat16), (P, P)).copy()

    in_maps = []
    for c in range(n_cores):
        lo = c * nloc
        locfeat = feat_pad[lo:lo + nloc_pad]
        if locfeat.shape[0] < nloc_pad:
            locfeat = np.vstack([locfeat, np.zeros((nloc_pad - locfeat.shape[0], d), np.float32)])
        in_maps.append({
            "feat": feat_pad,
            "locfeat": np.ascontiguousarray(locfeat),
            "beta128": beta128,
            "iota128": iota128,
            "gidx": gidx_streams[c],
            "slotw": slot_streams[c],
        })

    res = run_bass_kernel_spmd(nc, in_maps, core_ids=list(range(n_cores)),
                               trace=cfg.get('trace', False))
    last_exec_ns = res.exec_time_ns

    out = np.empty((n, d), dtype=np.float32)
    for c in range(n_cores):
        lo = c * nloc
        hi = min(lo + nloc, n)
        out[lo:hi] = res.results[c]["out"][:hi - lo]
    return out


FULL_CFG = dict(n=100000, n_pad=100352, d=64, n_cores=8, nloc=12500,
                tiles=98, chunk=25088, nchunks=4)


def kernel(feat, beta, src, dst):
    return _run(feat, beta, src, dst, dict(FULL_CFG))
